# revision 37
# baseline (speedup 1.0000x reference)
"""Distributed Trainium2 kernel for AnomalyMoE k-NN retrieval.

reference:  q = l2norm(test[L,N,D]); g = l2norm(normal[L,M,D])
            sim[l,n,m] = q . g ; out = (1 - mean_l max_m sim).reshape(1,1,16,16)

Strategy (8 NeuronCores):
- Shard gallery along M (6400 rows/core). Host pre-transposes each shard to
  [L, D, M_shard] (fp8e4m3) so every device DMA is dense; queries ship in
  both layouts ([L,D,N] for matmul weights, [L,N,D] for norms).
- Per core: dot[n,m] accumulated on TensorE with fp8 DoubleRow pair-matmuls
  (contraction 256/instruction).  Gallery row norms via elementwise Square
  split across THREE engines (ACT + DVE + Pool/GpSimd) + ones-DoubleRow-
  matmul partition sum, then one ACT Abs_reciprocal_sqrt.
- sim scale + running per-layer max fused in ONE DVE pass per (super, cx):
  tensor_tensor_reduce: dst=(pm*invg), lmax = max(lmax, max_m dst).
- Queries are NOT normalized on the way in: 1/||q_n|| commutes with max
  over gallery; applied at the end.
- Per-core partial results ([128, 2L] layer-maxes + inv query norms) are
  DMA'd out; the cross-core max + mean over layers + 1-x happens on host
  (the gather/unshard step).  KERNEL_HOST_COMBINE=0 restores the on-device
  AllReduce(max) path.
"""

import os
import sys
from concurrent.futures import ThreadPoolExecutor

sys.path.insert(0, "/opt/trn_rl_repo")

import numpy as np
import ml_dtypes

import concourse.bacc as bacc
import concourse.mybir as mybir
import concourse.tile as tile
from concourse.bass_utils import run_bass_kernel_spmd

F32 = mybir.dt.float32
BF16 = mybir.dt.bfloat16
AF = mybir.ActivationFunctionType
ALU = mybir.AluOpType
DR = mybir.MatmulPerfMode.DoubleRow

MODE = os.environ.get("KERNEL_MODE", "fp8")  # "fp8" | "bf16"
if MODE == "fp8":
    DT_IN = mybir.dt.float8e4
    NP_IN = ml_dtypes.float8_e4m3fn
else:
    DT_IN = mybir.dt.bfloat16
    NP_IN = ml_dtypes.bfloat16

NCORES = 8
L = 4
D = 1024
N = 256
M_FULL = 51200
MS = M_FULL // NCORES  # 6400 per core
KC = D // 128  # 8 contraction chunks of 128
KP = KC // 2  # 4 DoubleRow pairs
SUPER = 512
SUPERS = [(m0, min(SUPER, MS - m0)) for m0 in range(0, MS, SUPER)]  # 12x512 + 256
SKEW = int(os.environ.get("KERNEL_SKEW", "1"))  # software-pipeline depth
HOST_COMBINE = os.environ.get("KERNEL_HOST_COMBINE", "1") == "1"
# squares split per super: cycled list of (n_act, n_dve, n_pool) chunk counts
SQ_PAT = [
    tuple(int(x) for x in grp.split(","))
    for grp in os.environ.get("KERNEL_SQ", "5,3,0|5,2,1").split("|")
]
QN_EARLY = os.environ.get("KERNEL_QN_EARLY", "1") == "1"
QN_DVE = os.environ.get("KERNEL_QN_DVE", "0") == "1"
BUFS_G = int(os.environ.get("KERNEL_BUFS_G", "4"))
BUFS_SIM = int(os.environ.get("KERNEL_BUFS_SIM", "3"))
BUFS_PM = int(os.environ.get("KERNEL_BUFS_PM", "3"))
KERNEL_TAG = os.environ.get("KERNEL_TAG", "")
NEG = -3.0e38


def build():
    nc = bacc.Bacc("TRN2", target_bir_lowering=False, debug=False, num_devices=NCORES)
    g_ext = nc.dram_tensor("g_t", [L, D, MS], DT_IN, kind="ExternalInput")
    qt_ext = nc.dram_tensor("q_t", [L, D, N], DT_IN, kind="ExternalInput")
    qn_ext = nc.dram_tensor("q_n", [L, N, D], DT_IN, kind="ExternalInput")
    if HOST_COMBINE:
        lmax_ext = nc.dram_tensor("out_lmax", [128, 2 * L], F32, kind="ExternalOutput")
        invq_ext = nc.dram_tensor("out_invq", [128, 2 * L], F32, kind="ExternalOutput")
        out_ext = cc_in = cc_out = None
    else:
        out_ext = nc.dram_tensor("out", [2, 128], F32, kind="ExternalOutput")
        cc_in = nc.dram_tensor("cc_in", [2 * L, 128], F32)
        cc_out = nc.dram_tensor("cc_out", [2 * L, 128], F32, addr_space="Shared")
        lmax_ext = invq_ext = None

    with tile.TileContext(nc) as tc:
        with (
            tc.tile_pool(name="persist", bufs=1) as pp,
            tc.tile_pool(name="gsup", bufs=BUFS_G) as gpool,
            tc.tile_pool(name="sqp", bufs=BUFS_G) as sqpool,
            tc.tile_pool(name="invgp", bufs=3) as invgpool,
            tc.tile_pool(name="simp", bufs=BUFS_SIM) as simpool,
            tc.tile_pool(name="qsqp", bufs=2) as qsqpool,
            tc.tile_pool(name="pmp", bufs=BUFS_PM, space="PSUM") as pmpool,
            tc.tile_pool(name="pnorm", bufs=8 - 2 * BUFS_PM, space="PSUM") as pnormpool,
        ):
            # ---- persistent tiles ----
            # (qt DMA is issued inside the main loop, after the first gallery
            # super, so the first squares aren't queued behind the 1MB load)
            qt_sb = pp.tile([128, L * KC, N], DT_IN, name="qt_sb")
            qn_sb = pp.tile([128, 2 * L, D], DT_IN, name="qn_sb")
            if MODE == "fp8":
                ones_sb = pp.tile([128, 2, 128], DT_IN, name="ones_sb")
            else:
                ones_sb = pp.tile([128, 128], DT_IN, name="ones_sb")
            nc.gpsimd.memset(ones_sb[:], 1.0)
            qss = pp.tile([128, 2 * L], F32, name="qss")
            invq = pp.tile([128, 2 * L], F32, name="invq")
            # warm up ACT function tables (Square + Abs_reciprocal_sqrt)
            # while the first gallery DMA is in flight
            warm = pp.tile([128, 3], F32, name="warm")
            nc.vector.memset(warm[:, 0:1], 1.0)
            nc.scalar.activation(warm[:, 1:2], warm[:, 0:1], AF.Square)
            nc.scalar.activation(
                warm[:, 2:3], warm[:, 0:1], AF.Abs_reciprocal_sqrt
            )
            lmax_sb = pp.tile([128, 2 * L], F32, name="lmax_sb")
            nc.gpsimd.memset(lmax_sb[:], NEG)
            runmax = pp.tile([128, 2 * L, SUPER], BF16, name="runmax")
            # no memset needed: the first super of each layer copies into
            # runmax instead of maxing (see stage_b); tail columns beyond the
            # last super's msz are never read by the layer reduce (msz-aware)
            gmax_sb = pp.tile([128, 2 * L], F32, name="gmax_sb")
            smax_sb = pp.tile([128, 2 * L], F32, name="smax_sb")
            res_sb = pp.tile([128, 2], F32, name="res_sb")
            if KERNEL_TAG:
                # cache-buster: changes the BIR so stale NEFF caches miss
                tag_sb = pp.tile([128, 1], F32, name=f"tag_{KERNEL_TAG}")
                nc.gpsimd.memset(tag_sb[:], 1.0)

            def emit_q_norm_step(step):
                # spread the query-norm work through the main loop to fill
                # ACT bubbles; step 0 = DMA, 1..2L = squares, 2L+1 = rsqrt
                if step == 0:
                    nc.sync.dma_start(
                        qn_sb[:],
                        qn_ext.ap().rearrange("l (c p) d -> p (l c) d", p=128),
                    )
                elif step <= 2 * L:
                    j = step - 1
                    qsq_scr = qsqpool.tile([128, D], BF16, name="qsq_scr")
                    if QN_DVE:
                        # qn squares on DVE: out = (qn bypass 1) mult qn,
                        # accum_out = sum over free = ||q||^2
                        nc.vector.scalar_tensor_tensor(
                            out=qsq_scr[:],
                            in0=qn_sb[:, j, :],
                            scalar=1.0,
                            in1=qn_sb[:, j, :],
                            op0=ALU.bypass,
                            op1=ALU.mult,
                            accum_out=qss[:, j : j + 1],
                        )
                    else:
                        nc.scalar.activation(
                            qsq_scr[:],
                            qn_sb[:, j, :],
                            AF.Square,
                            accum_out=qss[:, j : j + 1],
                        )
                elif step == 2 * L + 1:
                    # invq = 1/sqrt(qss)
                    nc.scalar.activation(invq[:], qss[:], AF.Abs_reciprocal_sqrt)
                    if HOST_COMBINE:
                        nc.sync.dma_start(invq_ext.ap(), invq[:])

            # ---- main loop over layers and m-supers, software-pipelined ----
            # Stage A (dma + squares) runs SKEW supers ahead of stage B
            # (norm-mm, rsqrt, main-mms, epilogue) so the in-order ACT queue
            # never head-of-line-blocks a square behind an rsqrt.

            def stage_a(lx, m0, msz, sidx):
                gsup = gpool.tile([128, KC, SUPER], DT_IN, name="gsup")
                nc.sync.dma_start(
                    gsup[:, :, :msz],
                    g_ext.ap()[lx].rearrange("(k p) m -> p k m", p=128)[
                        :, :, m0 : m0 + msz
                    ],
                )
                sq = sqpool.tile([128, KC, SUPER], DT_IN, name="sq")
                if sidx >= len(work) - 3:
                    # tail supers: keep DVE light so its queue drains with the
                    # last matmuls instead of after them
                    n_act, n_dve, n_pool = (6, 1, 1)
                else:
                    n_act, n_dve, n_pool = SQ_PAT[sidx % len(SQ_PAT)]
                assert n_act + n_dve + n_pool == KC
                k = 0
                # ACT portion: one batched instruction (amortize SBUF access)
                if n_act:
                    ksl = slice(k, k + n_act)
                    nc.scalar.activation(
                        sq[:, ksl, :msz], gsup[:, ksl, :msz], AF.Square
                    )
                    k += n_act
                if n_dve:
                    ksl = slice(k, k + n_dve)
                    nc.vector.tensor_tensor(
                        out=sq[:, ksl, :msz],
                        in0=gsup[:, ksl, :msz],
                        in1=gsup[:, ksl, :msz],
                        op=ALU.mult,
                    )
                    k += n_dve
                if n_pool:
                    ksl = slice(k, k + n_pool)
                    nc.gpsimd.tensor_tensor(
                        out=sq[:, ksl, :msz],
                        in0=gsup[:, ksl, :msz],
                        in1=gsup[:, ksl, :msz],
                        op=ALU.mult,
                    )
                return gsup, sq

            def stage_b(lx, m0, msz, init_w, gsup, sq):
                # main matmuls first: they only need gsup+qt (not sq), so the
                # PE can start while this super's squares are still computing
                pm = pmpool.tile([128, 2, SUPER], F32, name="pm")
                for cx in range(2):
                    if MODE == "fp8":
                        for j in range(KP):
                            nc.tensor.matmul(
                                pm[:, cx, :msz],
                                qt_sb[
                                    :,
                                    lx * KC + 2 * j : lx * KC + 2 * j + 2,
                                    cx * 128 : (cx + 1) * 128,
                                ],
                                gsup[:, 2 * j : 2 * j + 2, :msz],
                                start=(j == 0),
                                stop=(j == KP - 1),
                                perf_mode=DR,
                            )
                    else:
                        for k in range(KC):
                            nc.tensor.matmul(
                                pm[:, cx, :msz],
                                qt_sb[:, lx * KC + k, cx * 128 : (cx + 1) * 128],
                                gsup[:, k, :msz],
                                start=(k == 0),
                                stop=(k == KC - 1),
                            )
                # gallery norms: pnorm[p, m] = sum_d g[d, m]^2 (all rows equal)
                pnorm = pnormpool.tile([128, SUPER], F32, name="pnorm")
                if MODE == "fp8":
                    for j in range(KP):
                        nc.tensor.matmul(
                            pnorm[:, :msz],
                            ones_sb[:],
                            sq[:, 2 * j : 2 * j + 2, :msz],
                            start=(j == 0),
                            stop=(j == KP - 1),
                            perf_mode=DR,
                        )
                else:
                    for k in range(KC):
                        nc.tensor.matmul(
                            pnorm[:, :msz],
                            ones_sb[:],
                            sq[:, k, :msz],
                            start=(k == 0),
                            stop=(k == KC - 1),
                        )
                # invg = 1/sqrt(pnorm) on ACT
                invg = invgpool.tile([128, SUPER], F32, name="invg")
                nc.scalar.activation(
                    invg[:, :msz], pnorm[:, :msz], AF.Abs_reciprocal_sqrt
                )
                # one DVE mult + one running max over both chunks at once
                invg_b = invg[:, :msz].rearrange(
                    "p (x m) -> p x m", x=1
                ).broadcast_to([128, 2, msz])
                sim = simpool.tile([128, 2, SUPER], BF16, name="sim")
                nc.vector.tensor_tensor(
                    out=sim[:, :, :msz],
                    in0=pm[:, :, :msz],
                    in1=invg_b,
                    op=ALU.mult,
                )
                # running elementwise max (bf16 SBUF = DVE 2x mode); final
                # 512->1 reduce happens once per layer.  runmax columns are
                # initialized by copy the first time each column range is
                # touched (init_w = columns already valid; no memset needed).
                j0 = 2 * lx
                w = min(init_w, msz)
                if w > 0:
                    rm = runmax[:, j0 : j0 + 2, :w]
                    nc.vector.tensor_tensor(
                        out=rm, in0=rm, in1=sim[:, :, :w], op=ALU.max
                    )
                if msz > w:
                    nc.vector.tensor_copy(
                        out=runmax[:, j0 : j0 + 2, w:msz],
                        in_=sim[:, :, w:msz],
                    )

            def layer_done(lx):
                # finalize this layer's local maxes (one [128,2,512] reduce)
                j = lx * 2
                nc.vector.reduce_max(
                    lmax_sb[:, j : j + 2],
                    runmax[:, j : j + 2, :],
                    axis=mybir.AxisListType.X,
                )
                if HOST_COMBINE:
                    # ship this layer's local maxes right away
                    nc.sync.dma_start(
                        lmax_ext.ap()[:, 2 * lx : 2 * lx + 2],
                        lmax_sb[:, 2 * lx : 2 * lx + 2],
                    )

            # layer 0 starts with a small super so the first DMA + squares +
            # matmul chain is short (compresses the pipeline-fill head).
            # init_w tracks how many runmax columns are valid per layer.
            SUPERS_L0 = [(0, 128), (128, 384)] + SUPERS[1:]
            work = []
            for lx in range(L):
                w = 0
                for m0, msz in SUPERS_L0 if lx == 0 else SUPERS:
                    work.append((lx, m0, msz, w))
                    w = max(w, msz)
            pending = []
            done_lx = 0
            # query-norm schedule: early (spread through the loop so the
            # invq chain never sits on the serial tail) or late (after loop)
            if QN_EARLY:
                qn_sched = {2: 0}
                for j in range(2 * L + 1):
                    qn_sched[6 + 4 * j] = j + 1
            else:
                qn_sched = {len(work) - 3: 0}
            for sidx, (lx, m0, msz, iw) in enumerate(work):
                pending.append((lx, m0, msz, iw) + stage_a(lx, m0, msz, sidx))
                if sidx == 0:
                    # only layer 0's query slice is needed immediately
                    nc.sync.dma_start(
                        qt_sb[:, :KC, :],
                        qt_ext.ap()[0:1].rearrange(
                            "l (k p) n -> p (l k) n", p=128
                        ),
                    )
                elif sidx == 2:
                    nc.sync.dma_start(
                        qt_sb[:, KC:, :],
                        qt_ext.ap()[1:L].rearrange(
                            "l (k p) n -> p (l k) n", p=128
                        ),
                    )
                if sidx in qn_sched:
                    emit_q_norm_step(qn_sched[sidx])
                if len(pending) > SKEW:
                    stage_b(*pending.pop(0))
                    nxt = pending[0][0] if pending else L
                    while done_lx < nxt:
                        layer_done(done_lx)
                        done_lx += 1
            while pending:
                stage_b(*pending.pop(0))
                nxt = pending[0][0] if pending else L
                while done_lx < nxt:
                    layer_done(done_lx)
                    done_lx += 1
            if not QN_EARLY:
                for qn_step in range(1, 2 * L + 2):
                    emit_q_norm_step(qn_step)

            if not HOST_COMBINE:
                nc.gpsimd.dma_start(
                    cc_in.ap().rearrange("c p -> p c"), lmax_sb[:]
                )
                nc.gpsimd.collective_compute(
                    "AllReduce",
                    mybir.AluOpType.max,
                    replica_groups=[list(range(NCORES))],
                    ins=[cc_in.ap().opt()],
                    outs=[cc_out.ap().opt()],
                )
                nc.gpsimd.dma_start(
                    gmax_sb[:], cc_out.ap().rearrange("c p -> p c")
                )

                # ---- scale by 1/||q||, mean over layers, 1 - x ----
                nc.vector.tensor_tensor(
                    out=smax_sb[:],
                    in0=gmax_sb[:],
                    in1=invq[:],
                    op=mybir.AluOpType.mult,
                )
                for cx in range(2):
                    ssum = pp.tile([128, 1], F32, name=f"ssum{cx}")
                    nc.vector.reduce_sum(
                        ssum[:],
                        smax_sb[:, cx : 2 * L : 2],
                        axis=mybir.AxisListType.X,
                    )
                    # out = 1 - ssum/L
                    nc.scalar.activation(
                        res_sb[:, cx : cx + 1],
                        ssum[:],
                        AF.Copy,
                        bias=1.0,
                        scale=-1.0 / L,
                    )
                for cx in range(2):
                    nc.sync.dma_start(
                        out_ext.ap()[cx : cx + 1, :].rearrange("c p -> p c"),
                        res_sb[:, cx : cx + 1],
                    )

    nc.compile()
    return nc


_NC_CACHE = None


def _get_nc():
    global _NC_CACHE
    if _NC_CACHE is None:
        _NC_CACHE = build()
    return _NC_CACHE


def _prep_shard(g_lp, c):
    # [L, MS, D] slice -> [L, D, MS] contiguous
    sl = g_lp[:, c * MS : (c + 1) * MS, :]
    return np.ascontiguousarray(sl.transpose(0, 2, 1))


def _prep_inputs(test_patch_tokens, normal_patch_tokens):
    q = np.asarray(test_patch_tokens, dtype=np.float32)
    g = np.asarray(normal_patch_tokens, dtype=np.float32)
    qn_lp = q.astype(NP_IN)  # [L, N, D]
    qt_lp = np.ascontiguousarray(qn_lp.transpose(0, 2, 1))  # [L, D, N]
    g_lp = g.astype(NP_IN)  # [L, M, D]
    with ThreadPoolExecutor(NCORES) as ex:
        shards = list(ex.map(lambda c: _prep_shard(g_lp, c), range(NCORES)))
    return [
        {"g_t": shards[c], "q_t": qt_lp, "q_n": qn_lp} for c in range(NCORES)
    ]


def kernel(test_patch_tokens: np.ndarray, normal_patch_tokens: np.ndarray):
    in_maps = _prep_inputs(test_patch_tokens, normal_patch_tokens)
    nc = _get_nc()
    results = run_bass_kernel_spmd(nc, in_maps, core_ids=list(range(NCORES))).results
    if HOST_COMBINE:
        # combine per-shard partial results: global max over cores, then
        # 1/||q|| scale, mean over layers, 1-x (tiny: 8*[128,8] values)
        lmax = np.max(
            np.stack([results[c]["out_lmax"] for c in range(NCORES)]), axis=0
        )  # [128, 2L]: column j = layer*2 + chunk
        invq = results[0]["out_invq"]  # identical on all cores
        smax = lmax * invq
        test_sim = smax.reshape(128, L, 2).mean(axis=1)  # [128(p), 2(chunk)]
        out = 1.0 - test_sim.T.reshape(N)  # n = chunk*128 + p
        return out.astype(np.float32).reshape(1, 1, 16, 16)
    out = results[0]["out"].astype(np.float32).reshape(1, 1, 16, 16)
    return out


# revision 38
# speedup vs baseline: 1.0088x; 1.0088x over previous
"""Distributed Trainium2 kernel for AnomalyMoE k-NN retrieval.

reference:  q = l2norm(test[L,N,D]); g = l2norm(normal[L,M,D])
            sim[l,n,m] = q . g ; out = (1 - mean_l max_m sim).reshape(1,1,16,16)

Strategy (8 NeuronCores):
- Shard gallery along M (6400 rows/core). Host pre-transposes each shard to
  [L, D, M_shard] (fp8e4m3) so every device DMA is dense; queries ship in
  both layouts ([L,D,N] for matmul weights, [L,N,D] for norms).
- Per core: dot[n,m] accumulated on TensorE with fp8 DoubleRow pair-matmuls
  (contraction 256/instruction).  Gallery row norms via elementwise Square
  split across THREE engines (ACT + DVE + Pool/GpSimd) + ones-DoubleRow-
  matmul partition sum, then one ACT Abs_reciprocal_sqrt.
- sim scale + running per-layer max fused in ONE DVE pass per (super, cx):
  tensor_tensor_reduce: dst=(pm*invg), lmax = max(lmax, max_m dst).
- Queries are NOT normalized on the way in: 1/||q_n|| commutes with max
  over gallery; applied at the end.
- Per-core partial results ([128, 2L] layer-maxes + inv query norms) are
  DMA'd out; the cross-core max + mean over layers + 1-x happens on host
  (the gather/unshard step).  KERNEL_HOST_COMBINE=0 restores the on-device
  AllReduce(max) path.
"""

import os
import sys
from concurrent.futures import ThreadPoolExecutor

sys.path.insert(0, "/opt/trn_rl_repo")

import numpy as np
import ml_dtypes

import concourse.bacc as bacc
import concourse.mybir as mybir
import concourse.tile as tile
from concourse.bass_utils import run_bass_kernel_spmd

F32 = mybir.dt.float32
BF16 = mybir.dt.bfloat16
AF = mybir.ActivationFunctionType
ALU = mybir.AluOpType
DR = mybir.MatmulPerfMode.DoubleRow

MODE = os.environ.get("KERNEL_MODE", "fp8")  # "fp8" | "bf16"
if MODE == "fp8":
    DT_IN = mybir.dt.float8e4
    NP_IN = ml_dtypes.float8_e4m3fn
else:
    DT_IN = mybir.dt.bfloat16
    NP_IN = ml_dtypes.bfloat16

NCORES = 8
L = 4
D = 1024
N = 256
M_FULL = 51200
MS = M_FULL // NCORES  # 6400 per core
KC = D // 128  # 8 contraction chunks of 128
KP = KC // 2  # 4 DoubleRow pairs
SUPER = 512
SUPERS = [(m0, min(SUPER, MS - m0)) for m0 in range(0, MS, SUPER)]  # 12x512 + 256
SKEW = int(os.environ.get("KERNEL_SKEW", "1"))  # software-pipeline depth
HOST_COMBINE = os.environ.get("KERNEL_HOST_COMBINE", "1") == "1"
# squares split per super: cycled list of (n_act, n_dve, n_pool) chunk counts
SQ_PAT = [
    tuple(int(x) for x in grp.split(","))
    for grp in os.environ.get("KERNEL_SQ", "5,3,0|5,2,1").split("|")
]
QN_EARLY = os.environ.get("KERNEL_QN_EARLY", "1") == "1"
QN_DVE = os.environ.get("KERNEL_QN_DVE", "0") == "1"
BUFS_G = int(os.environ.get("KERNEL_BUFS_G", "4"))
BUFS_SIM = int(os.environ.get("KERNEL_BUFS_SIM", "3"))
BUFS_PM = int(os.environ.get("KERNEL_BUFS_PM", "3"))
KERNEL_TAG = os.environ.get("KERNEL_TAG", "")
NEG = -3.0e38


def build():
    nc = bacc.Bacc("TRN2", target_bir_lowering=False, debug=False, num_devices=NCORES)
    g_ext = nc.dram_tensor("g_t", [L, D, MS], DT_IN, kind="ExternalInput")
    qt_ext = nc.dram_tensor("q_t", [L, D, N], DT_IN, kind="ExternalInput")
    qn_ext = nc.dram_tensor("q_n", [L, N, D], DT_IN, kind="ExternalInput")
    if HOST_COMBINE:
        lmax_ext = nc.dram_tensor("out_lmax", [128, 2 * L], F32, kind="ExternalOutput")
        invq_ext = nc.dram_tensor("out_invq", [128, 2 * L], F32, kind="ExternalOutput")
        out_ext = cc_in = cc_out = None
    else:
        out_ext = nc.dram_tensor("out", [2, 128], F32, kind="ExternalOutput")
        cc_in = nc.dram_tensor("cc_in", [2 * L, 128], F32)
        cc_out = nc.dram_tensor("cc_out", [2 * L, 128], F32, addr_space="Shared")
        lmax_ext = invq_ext = None

    with tile.TileContext(nc) as tc:
        with (
            tc.tile_pool(name="persist", bufs=1) as pp,
            tc.tile_pool(name="gsup", bufs=BUFS_G) as gpool,
            tc.tile_pool(name="sqp", bufs=BUFS_G) as sqpool,
            tc.tile_pool(name="invgp", bufs=3) as invgpool,
            tc.tile_pool(name="simp", bufs=BUFS_SIM) as simpool,
            tc.tile_pool(name="qsqp", bufs=2) as qsqpool,
            tc.tile_pool(name="pmp", bufs=BUFS_PM, space="PSUM") as pmpool,
            tc.tile_pool(name="pnorm", bufs=8 - 2 * BUFS_PM, space="PSUM") as pnormpool,
        ):
            # ---- persistent tiles ----
            # (qt DMA is issued inside the main loop, after the first gallery
            # super, so the first squares aren't queued behind the 1MB load)
            qt_sb = pp.tile([128, L * KC, N], DT_IN, name="qt_sb")
            qn_sb = pp.tile([128, 2 * L, D], DT_IN, name="qn_sb")
            if MODE == "fp8":
                ones_sb = pp.tile([128, 2, 128], DT_IN, name="ones_sb")
            else:
                ones_sb = pp.tile([128, 128], DT_IN, name="ones_sb")
            nc.gpsimd.memset(ones_sb[:], 1.0)
            qss = pp.tile([128, 2 * L], F32, name="qss")
            invq = pp.tile([128, 2 * L], F32, name="invq")
            # warm up ACT function tables (Square + Abs_reciprocal_sqrt)
            # while the first gallery DMA is in flight
            warm = pp.tile([128, 3], F32, name="warm")
            nc.vector.memset(warm[:, 0:1], 1.0)
            nc.scalar.activation(warm[:, 1:2], warm[:, 0:1], AF.Square)
            nc.scalar.activation(
                warm[:, 2:3], warm[:, 0:1], AF.Abs_reciprocal_sqrt
            )
            lmax_sb = pp.tile([128, 2 * L], F32, name="lmax_sb")
            nc.gpsimd.memset(lmax_sb[:], NEG)
            runmax = pp.tile([128, 2 * L, SUPER], BF16, name="runmax")
            # no memset needed: the first super of each layer copies into
            # runmax instead of maxing (see stage_b); tail columns beyond the
            # last super's msz are never read by the layer reduce (msz-aware)
            gmax_sb = pp.tile([128, 2 * L], F32, name="gmax_sb")
            smax_sb = pp.tile([128, 2 * L], F32, name="smax_sb")
            res_sb = pp.tile([128, 2], F32, name="res_sb")
            if KERNEL_TAG:
                # cache-buster: changes the BIR so stale NEFF caches miss
                tag_sb = pp.tile([128, 1], F32, name=f"tag_{KERNEL_TAG}")
                nc.gpsimd.memset(tag_sb[:], 1.0)

            def emit_q_norm_step(step):
                # spread the query-norm work through the main loop to fill
                # ACT bubbles; step 0 = DMA, 1..2L = squares, 2L+1 = rsqrt
                if step == 0:
                    nc.sync.dma_start(
                        qn_sb[:],
                        qn_ext.ap().rearrange("l (c p) d -> p (l c) d", p=128),
                    )
                elif step <= 2 * L:
                    j = step - 1
                    qsq_scr = qsqpool.tile([128, D], BF16, name="qsq_scr")
                    if QN_DVE:
                        # qn squares on DVE: out = (qn bypass 1) mult qn,
                        # accum_out = sum over free = ||q||^2
                        nc.vector.scalar_tensor_tensor(
                            out=qsq_scr[:],
                            in0=qn_sb[:, j, :],
                            scalar=1.0,
                            in1=qn_sb[:, j, :],
                            op0=ALU.bypass,
                            op1=ALU.mult,
                            accum_out=qss[:, j : j + 1],
                        )
                    else:
                        nc.scalar.activation(
                            qsq_scr[:],
                            qn_sb[:, j, :],
                            AF.Square,
                            accum_out=qss[:, j : j + 1],
                        )
                elif step == 2 * L + 1:
                    # invq = 1/sqrt(qss)
                    nc.scalar.activation(invq[:], qss[:], AF.Abs_reciprocal_sqrt)
                    if HOST_COMBINE:
                        nc.sync.dma_start(invq_ext.ap(), invq[:])

            # ---- main loop over layers and m-supers, software-pipelined ----
            # Stage A (dma + squares) runs SKEW supers ahead of stage B
            # (norm-mm, rsqrt, main-mms, epilogue) so the in-order ACT queue
            # never head-of-line-blocks a square behind an rsqrt.

            def stage_a(lx, m0, msz, sidx):
                gsup = gpool.tile([128, KC, SUPER], DT_IN, name="gsup")
                nc.sync.dma_start(
                    gsup[:, :, :msz],
                    g_ext.ap()[lx].rearrange("(k p) m -> p k m", p=128)[
                        :, :, m0 : m0 + msz
                    ],
                )
                sq = sqpool.tile([128, KC, SUPER], DT_IN, name="sq")
                if sidx >= len(work) - 3:
                    # tail supers: keep DVE light so its queue drains with the
                    # last matmuls instead of after them
                    n_act, n_dve, n_pool = (6, 1, 1)
                else:
                    n_act, n_dve, n_pool = SQ_PAT[sidx % len(SQ_PAT)]
                assert n_act + n_dve + n_pool == KC
                k = 0
                # ACT portion: one batched instruction (amortize SBUF access)
                if n_act:
                    ksl = slice(k, k + n_act)
                    nc.scalar.activation(
                        sq[:, ksl, :msz], gsup[:, ksl, :msz], AF.Square
                    )
                    k += n_act
                if n_dve:
                    ksl = slice(k, k + n_dve)
                    nc.vector.tensor_tensor(
                        out=sq[:, ksl, :msz],
                        in0=gsup[:, ksl, :msz],
                        in1=gsup[:, ksl, :msz],
                        op=ALU.mult,
                    )
                    k += n_dve
                if n_pool:
                    ksl = slice(k, k + n_pool)
                    nc.gpsimd.tensor_tensor(
                        out=sq[:, ksl, :msz],
                        in0=gsup[:, ksl, :msz],
                        in1=gsup[:, ksl, :msz],
                        op=ALU.mult,
                    )
                return gsup, sq

            def stage_b(lx, m0, msz, init_w, gsup, sq):
                # main matmuls first: they only need gsup+qt (not sq), so the
                # PE can start while this super's squares are still computing
                pm = pmpool.tile([128, 2, SUPER], F32, name="pm")
                for cx in range(2):
                    if MODE == "fp8":
                        for j in range(KP):
                            nc.tensor.matmul(
                                pm[:, cx, :msz],
                                qt_sb[
                                    :,
                                    lx * KC + 2 * j : lx * KC + 2 * j + 2,
                                    cx * 128 : (cx + 1) * 128,
                                ],
                                gsup[:, 2 * j : 2 * j + 2, :msz],
                                start=(j == 0),
                                stop=(j == KP - 1),
                                perf_mode=DR,
                            )
                    else:
                        for k in range(KC):
                            nc.tensor.matmul(
                                pm[:, cx, :msz],
                                qt_sb[:, lx * KC + k, cx * 128 : (cx + 1) * 128],
                                gsup[:, k, :msz],
                                start=(k == 0),
                                stop=(k == KC - 1),
                            )
                # gallery norms: pnorm[p, m] = sum_d g[d, m]^2 (all rows equal)
                pnorm = pnormpool.tile([128, SUPER], F32, name="pnorm")
                if MODE == "fp8":
                    for j in range(KP):
                        nc.tensor.matmul(
                            pnorm[:, :msz],
                            ones_sb[:],
                            sq[:, 2 * j : 2 * j + 2, :msz],
                            start=(j == 0),
                            stop=(j == KP - 1),
                            perf_mode=DR,
                        )
                else:
                    for k in range(KC):
                        nc.tensor.matmul(
                            pnorm[:, :msz],
                            ones_sb[:],
                            sq[:, k, :msz],
                            start=(k == 0),
                            stop=(k == KC - 1),
                        )
                # invg = 1/sqrt(pnorm) on ACT
                invg = invgpool.tile([128, SUPER], F32, name="invg")
                nc.scalar.activation(
                    invg[:, :msz], pnorm[:, :msz], AF.Abs_reciprocal_sqrt
                )
                # one DVE mult + one running max over both chunks at once
                invg_b = invg[:, :msz].rearrange(
                    "p (x m) -> p x m", x=1
                ).broadcast_to([128, 2, msz])
                sim = simpool.tile([128, 2, SUPER], BF16, name="sim")
                nc.vector.tensor_tensor(
                    out=sim[:, :, :msz],
                    in0=pm[:, :, :msz],
                    in1=invg_b,
                    op=ALU.mult,
                )
                # running elementwise max (bf16 SBUF = DVE 2x mode); final
                # 512->1 reduce happens once per layer.  runmax columns are
                # initialized by copy the first time each column range is
                # touched (init_w = columns already valid; no memset needed).
                j0 = 2 * lx
                w = min(init_w, msz)
                if w > 0:
                    rm = runmax[:, j0 : j0 + 2, :w]
                    nc.vector.tensor_tensor(
                        out=rm, in0=rm, in1=sim[:, :, :w], op=ALU.max
                    )
                if msz > w:
                    nc.vector.tensor_copy(
                        out=runmax[:, j0 : j0 + 2, w:msz],
                        in_=sim[:, :, w:msz],
                    )

            def layer_done(lx):
                # finalize this layer's local maxes (one [128,2,512] reduce)
                j = lx * 2
                nc.vector.reduce_max(
                    lmax_sb[:, j : j + 2],
                    runmax[:, j : j + 2, :],
                    axis=mybir.AxisListType.X,
                )
                if HOST_COMBINE:
                    # ship this layer's local maxes right away
                    nc.sync.dma_start(
                        lmax_ext.ap()[:, 2 * lx : 2 * lx + 2],
                        lmax_sb[:, 2 * lx : 2 * lx + 2],
                    )

            # init_w tracks how many runmax columns are valid per layer
            # (first super of a layer copies instead of maxing)
            work = []
            for lx in range(L):
                w = 0
                for m0, msz in SUPERS:
                    work.append((lx, m0, msz, w))
                    w = max(w, msz)
            pending = []
            done_lx = 0
            # query-norm schedule: early (spread through the loop so the
            # invq chain never sits on the serial tail) or late (after loop)
            if QN_EARLY:
                qn_sched = {2: 0}
                for j in range(2 * L + 1):
                    qn_sched[6 + 4 * j] = j + 1
            else:
                qn_sched = {len(work) - 3: 0}
            for sidx, (lx, m0, msz, iw) in enumerate(work):
                pending.append((lx, m0, msz, iw) + stage_a(lx, m0, msz, sidx))
                if sidx == 0:
                    # only layer 0's query slice is needed immediately
                    nc.sync.dma_start(
                        qt_sb[:, :KC, :],
                        qt_ext.ap()[0:1].rearrange(
                            "l (k p) n -> p (l k) n", p=128
                        ),
                    )
                elif sidx == 2:
                    nc.sync.dma_start(
                        qt_sb[:, KC:, :],
                        qt_ext.ap()[1:L].rearrange(
                            "l (k p) n -> p (l k) n", p=128
                        ),
                    )
                if sidx in qn_sched:
                    emit_q_norm_step(qn_sched[sidx])
                if len(pending) > SKEW:
                    stage_b(*pending.pop(0))
                    nxt = pending[0][0] if pending else L
                    while done_lx < nxt:
                        layer_done(done_lx)
                        done_lx += 1
            while pending:
                stage_b(*pending.pop(0))
                nxt = pending[0][0] if pending else L
                while done_lx < nxt:
                    layer_done(done_lx)
                    done_lx += 1
            if not QN_EARLY:
                for qn_step in range(1, 2 * L + 2):
                    emit_q_norm_step(qn_step)

            if not HOST_COMBINE:
                nc.gpsimd.dma_start(
                    cc_in.ap().rearrange("c p -> p c"), lmax_sb[:]
                )
                nc.gpsimd.collective_compute(
                    "AllReduce",
                    mybir.AluOpType.max,
                    replica_groups=[list(range(NCORES))],
                    ins=[cc_in.ap().opt()],
                    outs=[cc_out.ap().opt()],
                )
                nc.gpsimd.dma_start(
                    gmax_sb[:], cc_out.ap().rearrange("c p -> p c")
                )

                # ---- scale by 1/||q||, mean over layers, 1 - x ----
                nc.vector.tensor_tensor(
                    out=smax_sb[:],
                    in0=gmax_sb[:],
                    in1=invq[:],
                    op=mybir.AluOpType.mult,
                )
                for cx in range(2):
                    ssum = pp.tile([128, 1], F32, name=f"ssum{cx}")
                    nc.vector.reduce_sum(
                        ssum[:],
                        smax_sb[:, cx : 2 * L : 2],
                        axis=mybir.AxisListType.X,
                    )
                    # out = 1 - ssum/L
                    nc.scalar.activation(
                        res_sb[:, cx : cx + 1],
                        ssum[:],
                        AF.Copy,
                        bias=1.0,
                        scale=-1.0 / L,
                    )
                for cx in range(2):
                    nc.sync.dma_start(
                        out_ext.ap()[cx : cx + 1, :].rearrange("c p -> p c"),
                        res_sb[:, cx : cx + 1],
                    )

    nc.compile()
    return nc


_NC_CACHE = None


def _get_nc():
    global _NC_CACHE
    if _NC_CACHE is None:
        _NC_CACHE = build()
    return _NC_CACHE


def _prep_shard(g_lp, c):
    # [L, MS, D] slice -> [L, D, MS] contiguous
    sl = g_lp[:, c * MS : (c + 1) * MS, :]
    return np.ascontiguousarray(sl.transpose(0, 2, 1))


def _prep_inputs(test_patch_tokens, normal_patch_tokens):
    q = np.asarray(test_patch_tokens, dtype=np.float32)
    g = np.asarray(normal_patch_tokens, dtype=np.float32)
    qn_lp = q.astype(NP_IN)  # [L, N, D]
    qt_lp = np.ascontiguousarray(qn_lp.transpose(0, 2, 1))  # [L, D, N]
    g_lp = g.astype(NP_IN)  # [L, M, D]
    with ThreadPoolExecutor(NCORES) as ex:
        shards = list(ex.map(lambda c: _prep_shard(g_lp, c), range(NCORES)))
    return [
        {"g_t": shards[c], "q_t": qt_lp, "q_n": qn_lp} for c in range(NCORES)
    ]


def kernel(test_patch_tokens: np.ndarray, normal_patch_tokens: np.ndarray):
    in_maps = _prep_inputs(test_patch_tokens, normal_patch_tokens)
    nc = _get_nc()
    results = run_bass_kernel_spmd(nc, in_maps, core_ids=list(range(NCORES))).results
    if HOST_COMBINE:
        # combine per-shard partial results: global max over cores, then
        # 1/||q|| scale, mean over layers, 1-x (tiny: 8*[128,8] values)
        lmax = np.max(
            np.stack([results[c]["out_lmax"] for c in range(NCORES)]), axis=0
        )  # [128, 2L]: column j = layer*2 + chunk
        invq = results[0]["out_invq"]  # identical on all cores
        smax = lmax * invq
        test_sim = smax.reshape(128, L, 2).mean(axis=1)  # [128(p), 2(chunk)]
        out = 1.0 - test_sim.T.reshape(N)  # n = chunk*128 + p
        return out.astype(np.float32).reshape(1, 1, 16, 16)
    out = results[0]["out"].astype(np.float32).reshape(1, 1, 16, 16)
    return out
